# revision 7
# baseline (speedup 1.0000x reference)
"""Grouped-expert SwiGLU (MoE) kernel for Trainium2, expert-parallel over 8 cores.

Per core (one expert):
    g = x @ W_gate          [T, DOUT]
    u = x @ W_down          [T, DOUT]
    h = silu(g) * u
    out = h @ W_up          [T, DIN]

Layout strategy: compute transposed activations so every weight streams in
its natural HBM layout and the output lands in natural [token, din] layout.
  phase 0: xT[k] = transpose(x)          (PE transpose, bf16 eviction)
  phase 1: hT[j] = silu(Wg[:,j].T @ xT) * (Wd[:,j].T @ xT)   [dout, tokens]
  phase 2: out[m, :] = sum_j hT[j][:, m].T @ Wu[j, :]        [token, din]
Matmuls run in bf16 (fp32 PSUM accumulation); casts ride the PSUM evictions.
"""

import numpy as np

import concourse.bacc as bacc
import concourse.mybir as mybir
from concourse.tile import TileContext
from concourse.masks import make_identity
from concourse.bass_utils import run_bass_kernel_spmd

F32 = mybir.dt.float32
BF16 = mybir.dt.bfloat16
SILU = mybir.ActivationFunctionType.Silu
SIGMOID = mybir.ActivationFunctionType.Sigmoid
COPY = mybir.ActivationFunctionType.Copy

E = 8
T, DIN, DOUT = 2048, 2048, 1408
P = 128


def build_program(t=T, din=DIN, dout=DOUT, nstrip=512, sim_safe=False):
    kc = din // P   # contraction chunks for phase 1 (din)
    jc = dout // P  # dout blocks
    mc = t // P     # token blocks
    ns1 = t // nstrip    # token strips (phase 1)
    ns2 = din // nstrip  # din strips (phase 2)

    nc = bacc.Bacc(target_bir_lowering=False, trn_type="TRN2")
    x = nc.dram_tensor("x", [t, din], F32, kind="ExternalInput")
    wg = nc.dram_tensor("gate_proj", [din, dout], F32, kind="ExternalInput")
    wd = nc.dram_tensor("down_proj", [din, dout], F32, kind="ExternalInput")
    wu = nc.dram_tensor("up_proj", [dout, din], F32, kind="ExternalInput")
    out = nc.dram_tensor("out", [t, din], F32, kind="ExternalOutput")

    with TileContext(nc) as tc:
        with tc.tile_pool(name="persist", bufs=1) as persist:
            ident = persist.tile([P, P], F32, tag="ident", name="ident")
            make_identity(nc, ident)
            xT = [persist.tile([P, t], BF16, tag=f"xT{k}", name=f"xT{k}")
                  for k in range(kc)]
            hT = [persist.tile([P, t], BF16, tag=f"hT{j}", name=f"hT{j}")
                  for j in range(jc)]
            wub = [persist.tile([P, din], BF16, tag=f"wub{j}", name=f"wub{j}")
                   for j in range(jc)]

            # ---- phase 0: x -> xT (bf16) via PE transpose ----
            with tc.tile_pool(name="xstage", bufs=2) as xstage, \
                 tc.tile_pool(name="tpsum", bufs=4, space="PSUM") as tpsum:
                for m in range(mc):
                    xs = xstage.tile([P, din], F32, tag="xs", name="xs")
                    nc.sync.dma_start(out=xs, in_=x.ap()[m * P:(m + 1) * P, :])
                    for k in range(kc):
                        pt = tpsum.tile([P, P], F32, tag="pt", name="pt")
                        nc.tensor.transpose(pt, xs[:, k * P:(k + 1) * P], ident)
                        dst = xT[k][:, m * P:(m + 1) * P]
                        if k % 2 == 0:
                            nc.scalar.activation(dst, pt, COPY)
                        else:
                            nc.vector.tensor_copy(out=dst, in_=pt)

            # ---- phase 1: hT[j] = silu(gT) * uT; also cast Wu panels ----
            half = din // 2
            with tc.tile_pool(name="wstage", bufs=2) as wstage, \
                 tc.tile_pool(name="wbf", bufs=2) as wbf, \
                 tc.tile_pool(name="wustage", bufs=1) as wustage, \
                 tc.tile_pool(name="silu", bufs=3) as silu_pool, \
                 tc.tile_pool(name="gpsum", bufs=2, space="PSUM") as gpsum, \
                 tc.tile_pool(name="upsum", bufs=2, space="PSUM") as upsum:
                for j in range(jc):
                    wg_bf = wbf.tile([P, din], BF16, tag="wg_bf", name="wg_bf")
                    wd_bf = wbf.tile([P, din], BF16, tag="wd_bf", name="wd_bf")
                    for w_dram, w_bf, tg in ((wg, wg_bf, "g"), (wd, wd_bf, "d")):
                        for h in range(2):
                            st = wstage.tile([P, half], F32, tag=f"wst{tg}",
                                             name=f"wst{tg}{h}")
                            src = w_dram.ap()[h * half:(h + 1) * half,
                                              j * P:(j + 1) * P] \
                                .rearrange("(c p) n -> p c n", p=P)
                            dst = st.rearrange("p (c n) -> p c n", n=P)
                            nc.sync.dma_start(out=dst, in_=src)
                            nc.vector.tensor_copy(
                                out=w_bf[:, h * half:(h + 1) * half], in_=st)
                    # cast this j's Wu panel while PE runs phase-1 matmuls
                    wust = wustage.tile([P, din], F32, tag="wust", name="wust")
                    nc.sync.dma_start(out=wust, in_=wu.ap()[j * P:(j + 1) * P, :])
                    nc.vector.tensor_copy(out=wub[j], in_=wust)

                    for n in range(ns1):
                        tok = slice(n * nstrip, (n + 1) * nstrip)
                        pg = gpsum.tile([P, nstrip], F32, tag="pg", name="pg")
                        pu = upsum.tile([P, nstrip], F32, tag="pu", name="pu")
                        for k in range(kc):
                            nc.tensor.matmul(
                                pg, lhsT=wg_bf[:, k * P:(k + 1) * P],
                                rhs=xT[k][:, tok],
                                start=(k == 0), stop=(k == kc - 1))
                        for k in range(kc):
                            nc.tensor.matmul(
                                pu, lhsT=wd_bf[:, k * P:(k + 1) * P],
                                rhs=xT[k][:, tok],
                                start=(k == 0), stop=(k == kc - 1))
                        sl = silu_pool.tile([P, nstrip], BF16, tag="sl", name="sl")
                        if sim_safe:
                            # CoreSim has no Silu; silu(g) = g * sigmoid(g)
                            nc.scalar.activation(sl, pg, SIGMOID)
                            nc.vector.tensor_mul(out=sl, in0=sl, in1=pg)
                        else:
                            nc.scalar.activation(sl, pg, SILU)
                        nc.vector.tensor_mul(out=hT[j][:, tok], in0=sl, in1=pu)

            # ---- phase 2: out = hT.T @ Wu ----
            with tc.tile_pool(name="ostage", bufs=3) as ostage, \
                 tc.tile_pool(name="opsum", bufs=4, space="PSUM") as opsum:
                for m in range(mc):
                    for n in range(ns2):
                        dsl = slice(n * nstrip, (n + 1) * nstrip)
                        po = opsum.tile([P, nstrip], F32, tag="po", name="po")
                        for j in range(jc):
                            nc.tensor.matmul(
                                po, lhsT=hT[j][:, m * P:(m + 1) * P],
                                rhs=wub[j][:, dsl],
                                start=(j == 0), stop=(j == jc - 1))
                        ot = ostage.tile([P, nstrip], F32, tag="ot", name="ot")
                        if (m * ns2 + n) % 2 == 0:
                            nc.scalar.activation(ot, po, COPY)
                        else:
                            nc.vector.tensor_copy(out=ot, in_=po)
                        nc.sync.dma_start(
                            out=out.ap()[m * P:(m + 1) * P, dsl], in_=ot)

    nc.finalize()
    return nc


_program = None


def kernel(x, gate_proj, down_proj, up_proj):
    global _program
    if _program is None:
        _program = build_program()
    in_maps = [
        {
            "x": np.ascontiguousarray(x[e], dtype=np.float32),
            "gate_proj": np.ascontiguousarray(gate_proj[e], dtype=np.float32),
            "down_proj": np.ascontiguousarray(down_proj[e], dtype=np.float32),
            "up_proj": np.ascontiguousarray(up_proj[e], dtype=np.float32),
        }
        for e in range(E)
    ]
    res = run_bass_kernel_spmd(_program, in_maps, list(range(E)))
    return np.stack([res.results[e]["out"] for e in range(E)], axis=0)
